# revision 31
# baseline (speedup 1.0000x reference)
"""Trainium2 Bass kernel for nn_DerivedMLP (1,2,64,2,512,512) -> (1,64).

Computation (per the original nn.Module):
  x: (1, 2, 64, 2, 512, 512) f32; channel 0 of dim1 holds the [n, phi] fields.
  gamma[t] = -mean(n[t] * d(phi[t])/dy)        (numpy.gradient semantics on y)
  feats    = stack([input_derived, gamma])     -> (2, 64)
  out      = w2 @ gelu_tanh(w1 @ feats + b1) + b2   (1x1 convs over t)

Sharding: fully independent per time step t, so t is sharded across the 8
NeuronCores: core k handles t in [8k, 8k+8).  Zero communication; each core
streams a 16 MB slice of x (only channel 0 is read).  The host concatenates
the 8 per-core (1, 8) outputs.

Per-core kernel (Tile framework; the 16 MB HBM stream at ~360 GB/s is the
roofline, so everything else must hide under it or shrink the tail):
  - Load order ends p5 p6 p7 n5 n6(3 chunks) n7(7 shrinking chunks) so only
    one 128-column fused op remains after the last byte lands;
    DMA-completion semaphore visibility is +900 ns and dominates the tail.
  - DVE scalar_tensor_tensor fuses the product (n * d) AND the free-axis
    reduction into ONE op at the cost of a plain multiply, writing
    per-partition sums straight into acc columns; nothing else runs
    mid-stream (tensor_tensor_reduce would do the same but crashes the TRN2
    exec unit at runtime).
  - y-segment edge columns of each diff are fixed with strided sub+mul ops.
  - One big t6 chunk + one t7 chunk run product-on-GPSIMD + ACT Copy-accum
    reduce so DVE keeps pace with the final chunk arrivals.
  - Partition reduction + MLP layer 1 + both biases collapse into
    accumulating PE matmuls in a TRANSPOSED (t, h) layout: lhsT = acc bank,
    rhs = w1_gamma broadcast over 128 partitions (built on-chip by a
    ones-lhsT matmul so the staging DMA stays 8 partitions), plus one
    [derived; ones] x [w1_derived; b1] matmul.  Tail chunk columns land in
    8-wide acc banks folded by extra ~15 ns matmuls, ordered by expected
    completion.
  - gelu is one native Gelu_apprx_tanh ACT op (matches jax.nn.gelu
    approximate=True); layer 2 collapses to ONE fused DVE
    scalar_tensor_tensor against host-packed [w2, b2] rows with an ones
    column in h, accumulating out[t] directly - no second matmul, no PSUM
    copy.
  - All weights/derived ship as ONE (8, 21) staging DMA tucked into the SP
    FIFO behind the first two loads (~4 ns of stream).
"""

import os
import sys

import numpy as np

for _p in ("/opt/trn_rl_repo",):
    if os.path.isdir(_p) and _p not in sys.path:
        sys.path.insert(0, _p)

# Defensive: the bass execution path runs through the axon PJRT plugin; if the
# caller's env pinned JAX_PLATFORMS without axon (and jax isn't initialized
# yet), restore it so jax.devices() can see the NeuronCores.
if (
    os.environ.get("AXON_H4_ENABLED") == "1"
    or os.environ.get("AXON_TERMINAL_JOB_NAME")
) and "jax" not in sys.modules:
    _plat = os.environ.get("JAX_PLATFORMS", "")
    if _plat and "axon" not in _plat:
        os.environ["JAX_PLATFORMS"] = "axon," + _plat

# ---- problem constants (hardcoded per contract) ----
DX = 0.1
B, C, T, V, NX, NY = 1, 2, 64, 2, 512, 512
N_CORES = 8
T_PER_CORE = T // N_CORES  # 8
P = 128                    # SBUF partitions
FREE = (NX * NY) // P      # 2048 f32 per partition = whole 512x512 image
SEG = NY                   # 512; partition rows hold 4 y-segments each
GAMMA_SCALE = -(0.5 / DX) / float(NX * NY)

# tail chunking (columns of the 2048-wide image); engine per chunk:
# "dve" = one fused scalar_tensor_tensor; "pool" = gpsimd product + ACT
# Copy-accum reduce (tensor_tensor_reduce and gpsimd scalar_tensor_tensor
# don't survive the TRN2 runtime/codegen)
N6_CHUNKS = [768, 640, 640]                      # t6
N6_ENGINE = ["pool", "dve", "dve"]
N7_CHUNKS = [512, 512, 256, 240, 208, 192, 128]  # t7: shrinking tail
N7_ENGINE = ["dve", "dve", "pool", "dve", "dve", "dve", "dve"]
# chunk -> acc bank, ordered by expected completion so each in-order PE
# matmul is ready when reached and a late ACT reduce only delays one mm
N6_BANK = [2, 0, 1]
N7_BANK = [0, 1, 5, 2, 3, 4, 6]

# acc column banks of 8: bank 0 cols 0..5 = t0..t5; bank k's cols 8k+6/8k+7
# hold (t6 chunk k, t7 chunk k).  Full-8-wide banks keep every PE matmul
# writing PSUM partitions 0..7 (partition-offset PSUM writes are rejected);
# unused columns stay at the initial memset zero.
N_BANKS = max(len(N6_CHUNKS), len(N7_CHUNKS))
ACC_COLS = 8 * N_BANKS

# stage tile layout (8 x 21): row 0 of cols 0:4 = w1_gamma (prescaled);
# rows 0:2 of cols 4:8 = [w1_derived; b1]; rows 0:2 of cols 8:16 =
# [derived_t; ones]; rows 0:8 of cols 16:21 = [w2, b2] per row.  w1_gamma is
# broadcast to 128 partitions on-chip (ones-lhsT matmul) so the stage DMA
# stays 8 partitions (~4 ns of stream instead of ~60).
STAGE_ROWS = 8
STAGE_COLS = 21

_CACHE = {}


def _build_nc():
    import concourse.mybir as mybir
    import concourse.tile as tile
    import concourse.bass as bass
    from concourse import bacc

    f32 = mybir.dt.float32
    sub = mybir.AluOpType.subtract
    mult = mybir.AluOpType.mult
    Gelu = mybir.ActivationFunctionType.Gelu_apprx_tanh

    nc = bacc.Bacc(
        "TRN2", target_bir_lowering=False, debug=False, num_devices=N_CORES
    )

    xs = nc.dram_tensor("xs", (T_PER_CORE, 2, P, FREE), f32, kind="ExternalInput").ap()
    stage_d = nc.dram_tensor(
        "stage", (STAGE_ROWS, STAGE_COLS), f32, kind="ExternalInput"
    ).ap()
    out = nc.dram_tensor("out", (1, T_PER_CORE), f32, kind="ExternalOutput").ap()

    LAST2, LAST = T_PER_CORE - 2, T_PER_CORE - 1  # 6, 7

    with tile.TileContext(nc) as tc:
        with (
            tc.tile_pool(name="io", bufs=4) as io,
            tc.tile_pool(name="small", bufs=1) as small,
            tc.tile_pool(name="ps", bufs=1, space=bass.MemorySpace.PSUM) as ps,
        ):
            stage = small.tile([STAGE_ROWS, STAGE_COLS], f32)
            acc = small.tile([P, ACC_COLS], f32)
            h8 = small.tile([T_PER_CORE, 5], f32)
            j8 = small.tile([T_PER_CORE, 5], f32)
            res8 = small.tile([T_PER_CORE, 1], f32)
            warm = small.tile([1, 1], f32)
            onesr = small.tile([1, P], f32)
            w1gb = small.tile([P, 4], f32)

            nc.vector.memset(onesr[:], 1.0)
            nc.vector.memset(acc[:], 0.0)
            # whole-tile memset (partition-offset memset fails the BIR
            # verifier); gelu later overwrites cols 0:4, leaving the ones col
            nc.vector.memset(h8[:], 1.0)
            nc.vector.memset(warm[:], 0.0)
            # 1-wide dummy Gelu: hoists the ACT function-table load off the
            # kernel tail, overlapping it with the DMA stream
            nc.scalar.activation(warm[:], warm[:], Gelu, bias=0.0, scale=1.0)

            # ---- big loads on the SP ring ----
            # order: (p0 n0) .. (p4 n4) p5 p6 p7 n5 n6-chunks n7-chunks
            ptiles, ntiles = {}, {}

            def load_p(t):
                ptiles[t] = io.tile([P, FREE], f32, tag="p", name=f"p{t}")
                nc.sync.dma_start(ptiles[t][:], xs[t, 1])

            def load_n(t, chunks=None):
                ntiles[t] = io.tile([P, FREE], f32, tag="n", name=f"n{t}")
                if chunks is None:
                    nc.sync.dma_start(ntiles[t][:], xs[t, 0])
                else:
                    g = 0
                    for w in chunks:
                        nc.sync.dma_start(
                            ntiles[t][:, g : g + w], xs[t, 0][:, g : g + w]
                        )
                        g += w

            load_p(0)
            load_n(0)
            # tiny weights/derived DMA tucked into the SP FIFO behind the
            # first two 1 MB loads: ~4 ns of stream, needed at ~7 us
            nc.sync.dma_start(stage[:], stage_d[:])
            # broadcast w1g to all 128 partitions: ones-lhsT matmul + copy
            bc_ps = ps.tile([P, 4], f32)
            nc.tensor.matmul(bc_ps[:], onesr[:], stage[0:1, 0:4], start=True, stop=True)
            nc.vector.tensor_copy(w1gb[:], bc_ps[:])
            for t in range(1, LAST2 - 1):
                load_p(t)
                load_n(t)
            load_p(LAST2 - 1)  # p5
            load_p(LAST2)      # p6
            load_p(LAST)       # p7
            load_n(LAST2 - 1)  # n5
            load_n(LAST2, N6_CHUNKS)
            load_n(LAST, N7_CHUNKS)

            # ---- stencil: d = grad_y(phi) * 2dx (segment-local) ----
            dtiles = {}

            def make_diff(t):
                d = io.tile([P, FREE], f32, tag="d", name=f"d{t}")
                dtiles[t] = d
                ptile = ptiles[t]
                # interior central difference (incl. garbage at segment
                # seams, overwritten below)
                nc.vector.tensor_tensor(
                    d[:, 1 : FREE - 1], ptile[:, 2:FREE], ptile[:, 0 : FREE - 2], sub
                )
                # y-segment left edges: 2*(p[g+1]-p[g]); right: 2*(p[g]-p[g-1])
                # (tensor_tensor_reduce would fold the x2 but crashes the
                # TRN2 exec unit at runtime)
                nc.vector.tensor_tensor(
                    d[:, 0:FREE:SEG], ptile[:, 1:FREE:SEG], ptile[:, 0:FREE:SEG], sub
                )
                nc.vector.tensor_scalar_mul(d[:, 0:FREE:SEG], d[:, 0:FREE:SEG], 2.0)
                nc.vector.tensor_tensor(
                    d[:, SEG - 1 : FREE : SEG],
                    ptile[:, SEG - 1 : FREE : SEG],
                    ptile[:, SEG - 2 : FREE : SEG],
                    sub,
                )
                nc.vector.tensor_scalar_mul(
                    d[:, SEG - 1 : FREE : SEG], d[:, SEG - 1 : FREE : SEG], 2.0
                )

            def fused_full(t, col):
                # acc[:, col] = sum_y n*d in ONE DVE op (product written in
                # place over d; scalar_tensor_tensor's accum port does the
                # free-axis reduction for free)
                d, n = dtiles[t], ntiles[t]
                nc.vector.scalar_tensor_tensor(
                    d[:], n[:], 1.0, d[:], mult, mult,
                    accum_out=acc[:, col : col + 1],
                )

            Copy = mybir.ActivationFunctionType.Copy

            def fused_chunk(t, g, w, col, engine):
                d, n = dtiles[t], ntiles[t]
                if engine == "dve":
                    nc.vector.scalar_tensor_tensor(
                        d[:, g : g + w], n[:, g : g + w], 1.0, d[:, g : g + w],
                        mult, mult,
                        accum_out=acc[:, col : col + 1],
                    )
                else:
                    nc.gpsimd.tensor_tensor(
                        d[:, g : g + w], n[:, g : g + w], d[:, g : g + w], mult
                    )
                    nc.scalar.activation(
                        d[:, g : g + w], d[:, g : g + w], Copy, bias=0.0,
                        scale=1.0, accum_out=acc[:, col : col + 1],
                    )

            for t in range(LAST2):  # t0..t5: full-width diff + fused reduce
                make_diff(t)
                fused_full(t, t)
            make_diff(LAST2)
            make_diff(LAST)
            # tail chunks: banked acc columns (bank b cols 8b+6 / 8b+7)
            g = 0
            for k, w in enumerate(N6_CHUNKS):
                fused_chunk(LAST2, g, w, 8 * N6_BANK[k] + 6, N6_ENGINE[k])
                g += w
            g = 0
            for k, w in enumerate(N7_CHUNKS):
                fused_chunk(LAST, g, w, 8 * N7_BANK[k] + 7, N7_ENGINE[k])
                g += w

            # ---- partition reduction + MLP, fused into PE matmuls ----
            # Transposed layout: z8[t,h] so layer 2 becomes one DVE op.
            # z8[t,h] = derived[t]*w1d[h] + b1[h]          (mm_db, start)
            #         + sum_p acc[p, bank_cols]*w1g[h]     (one mm per bank)
            z8 = ps.tile([T_PER_CORE, 4], f32)
            nc.tensor.matmul(
                z8[:], stage[0:2, 8:16], stage[0:2, 4:8], start=True, stop=False
            )
            for k in range(N_BANKS):
                nc.tensor.matmul(
                    z8[:], acc[:, 8 * k : 8 * k + 8], w1gb[:],
                    start=False, stop=(k == N_BANKS - 1), skip_group_check=True,
                )
            # h8 = gelu_tanh(z8); col 4 of h8 stays ones (bias col)
            nc.scalar.activation(h8[:, 0:4], z8[:], Gelu, bias=0.0, scale=1.0)
            # out[t] = sum_h h8[t,h]*w2[h] + b2  -- one fused DVE op against
            # the host-packed [w2, b2] rows in stage
            nc.vector.scalar_tensor_tensor(
                j8[:], h8[:], 1.0, stage[0:T_PER_CORE, 16:21], mult, mult,
                accum_out=res8[:],
            )
            nc.sync.dma_start(out[:], res8[:])

    nc.compile()
    return nc


def get_nc():
    if "nc" not in _CACHE:
        _CACHE["nc"] = _build_nc()
    return _CACHE["nc"]


def make_in_maps(x, input_derived, w1, b1, w2, b2):
    x = np.asarray(x, dtype=np.float32)
    input_derived = np.asarray(input_derived, dtype=np.float32)
    w1 = np.asarray(w1, dtype=np.float32)   # (4, 2): cols = (derived, gamma)
    b1 = np.asarray(b1, dtype=np.float32)   # (4,)
    w2 = np.asarray(w2, dtype=np.float32)   # (1, 4)
    b2 = np.asarray(b2, dtype=np.float32)   # (1,)

    # feats order in the reference is (derived, gamma): w1[:,0] multiplies
    # derived, w1[:,1] multiplies gamma.  The kernel feeds raw stencil sums,
    # so the gamma column absorbs GAMMA_SCALE.
    w1g = w1[:, 1] * np.float32(GAMMA_SCALE)  # (4,)
    w1d = w1[:, 0]                            # (4,)

    x0 = x[0, 0]  # (64, 2, 512, 512): [t, v, nx, ny]
    in_maps = []
    for k in range(N_CORES):
        t0 = k * T_PER_CORE
        xs_k = np.ascontiguousarray(x0[t0 : t0 + T_PER_CORE]).reshape(
            T_PER_CORE, 2, P, FREE
        )
        stage = np.zeros((STAGE_ROWS, STAGE_COLS), dtype=np.float32)
        stage[0, 0:4] = w1g
        stage[0, 4:8] = w1d
        stage[1, 4:8] = b1
        stage[0, 8:16] = input_derived[0, t0 : t0 + T_PER_CORE]
        stage[1, 8:16] = 1.0
        stage[0:T_PER_CORE, 16:20] = w2[0][None, :]
        stage[0:T_PER_CORE, 20] = b2[0]
        in_maps.append({"xs": xs_k, "stage": stage})
    return in_maps


def kernel(x, input_derived, w1, b1, w2, b2, trace=False):
    import time

    from concourse.bass_utils import run_bass_kernel_spmd

    nc = get_nc()
    in_maps = make_in_maps(x, input_derived, w1, b1, w2, b2)
    for attempt in range(3):  # the axon PJRT path has rare transient INTERNALs
        try:
            results = run_bass_kernel_spmd(
                nc, in_maps, core_ids=list(range(N_CORES)), trace=trace
            )
            break
        except Exception:
            if attempt == 2:
                raise
            time.sleep(5.0)
    _CACHE["last_results"] = results
    return np.concatenate([r["out"] for r in results.results], axis=1)
